# revision 13
# baseline (speedup 1.0000x reference)
"""Trainium2 Bass kernel for nn_DGCRNN (ChebConv K=3 GNN, robot-node output).

Math: the reference returns only node 0 (robot) of the ChebConv output:
    out = r @ Ar + c1 @ W1 + c2 @ (2*W2) + cheb_b
with Ar = W0 - W2 + v1[0]*W1 + 2*v2[0]*W2, v1 = L_hat[0,:],
v2 = (L_hat @ L_hat)[0,:], c_k = sum_n v_k[n+1] * h_n over the 63 human-node
embeddings h_n, and r the robot embedding (both 2-layer relu MLPs).

Fast path (const-v): for the complete graph that setup_inputs() builds, every
human node has the same degree, so v1[1:] and v2[1:] are constant vectors.
Then c1 and c2 are both proportional to the plain segmented sum
S[f,b] = sum_n h2[f,b,n], and the entire v-weighting folds into one
host-computed combine matrix Wc = v1[1]*W1 + 2*v2[1]*W2:
    out = r @ Ar + S @ Wc + cheb_b
This removes the per-token v-multiply on device entirely. A general-v kernel
(the previous implementation) is kept as fallback for arbitrary graphs.

Sharding: pure data parallel over the batch dim (512 / 8 cores = 64 each).

Const-v implementation: bf16 datapath (fp32 PSUM accumulation, fp32 final
combine), 5 token chunks (8/16/16/16/8 batches) pipelined across engines:
PE does the MLP matmuls, ACT drains relu1 (+relu2 of the edge chunks), DVE
drains relu2 of the middle chunks and does the segmented reductions, GPSIMD
pre-folds 63->21 for two middle chunks so their reduce is cheap, SP issues
all DMAs (weights pack, fp32 pack, tokens, output).
"""

import numpy as np

B, N, F, HID = 512, 64, 64, 128
ROBOT_DIM, HUMAN_DIM = 9, 5
NCORES = 8
BL = B // NCORES      # 64 batches per core
NH = N - 1            # 63 human nodes
TOK = BL * NH         # 4032 human tokens per core

# const-v chunking: batch counts per chunk
CB = [8, 16, 16, 16, 8]
CSTART = [0, 8, 24, 40, 56]            # first batch of each chunk
CT = [c * NH for c in CB]              # tokens per chunk
CTS = [s * NH for s in CSTART]         # first token of each chunk
SL = 504                               # cols per matmul slice (<= 512 psum bank)

_STATE = {}


def _bf16(a):
    import ml_dtypes
    return np.asarray(a, np.float32).astype(ml_dtypes.bfloat16)


# --------------------------------------------------------------------------
# const-v kernel
# --------------------------------------------------------------------------

def _build_bass_const():
    import concourse.bass as bass
    from concourse import bacc, mybir

    f32 = mybir.dt.float32
    bf16 = mybir.dt.bfloat16
    AF = mybir.ActivationFunctionType
    ALU = mybir.AluOpType
    AX = mybir.AxisListType

    nc = bacc.Bacc("TRN2", target_bir_lowering=False, debug=False)

    # --- DRAM I/O ---
    d_hTb = nc.dram_tensor("hTb", [HUMAN_DIM + 1, TOK], bf16, kind="ExternalInput").ap()
    d_wp = nc.dram_tensor("wp", [HID, 576], bf16, kind="ExternalInput").ap()
    d_fp = nc.dram_tensor("fp", [HID, 2], f32, kind="ExternalInput").ap()
    d_out = nc.dram_tensor("out", [BL, F], f32, kind="ExternalOutput").ap()

    # --- SBUF ---
    hTb = nc.alloc_sbuf_tensor("hTb_sb", [HUMAN_DIM + 1, TOK], bf16).ap()
    wp = nc.alloc_sbuf_tensor("wp_sb", [HID, 576], bf16).ap()
    fp = nc.alloc_sbuf_tensor("fp_sb", [HID, 2], f32).ap()
    h1 = nc.alloc_sbuf_tensor("h1_sb", [HID, TOK], bf16).ap()
    h2d = nc.alloc_sbuf_tensor("h2d_sb", [F, BL, NH], bf16).ap()
    tmpa = nc.alloc_sbuf_tensor("tmpa_sb", [F, 48, 21], bf16).ap()
    tmpb = nc.alloc_sbuf_tensor("tmpb_sb", [F, 48, 21], bf16).ap()
    r1 = nc.alloc_sbuf_tensor("r1_sb", [HID, BL], bf16).ap()
    r2a = nc.alloc_sbuf_tensor("r2a_sb", [F + 1, BL], bf16).ap()
    c12 = nc.alloc_sbuf_tensor("c12_sb", [F, BL], bf16).ap()
    out_sb = nc.alloc_sbuf_tensor("out_sb", [BL, F], f32).ap()

    # weight-pack slices (bf16)
    wh1a = wp[0:HUMAN_DIM + 1, 0:128]     # L1 stationary [6, 128]
    wh2 = wp[:, 128:192]                  # L2 stationary [128, 64]
    wr1a = wp[0:ROBOT_DIM + 1, 192:320]   # robot L1 stationary [10, 128]
    rTa = wp[0:ROBOT_DIM + 1, 320:384]    # robot tokens [10, 64]
    wr2 = wp[:, 384:448]                  # robot L2 stationary [128, 64]
    Wc = wp[0:F, 448:512]                 # folded combine [64, 64]
    Ara = wp[0:F + 1, 512:576]            # robot combine + cheb_b row [65, 64]
    # fp32-pack slices (biases only)
    bh2 = fp[0:F, 0:1]                    # wh2 bias [64, 1]
    br2 = fp[0:F, 1:2]                    # robot L2 bias [64, 1]

    # --- PSUM: ph1 has 5 512-col slots (5 banks), ph2 has 3 (3 banks) ---
    ph1 = nc.alloc_psum_tensor("ph1", [HID, 2560], f32).ap()
    ph2 = nc.alloc_psum_tensor("ph2", [F, 1536], f32).ap()
    pr1 = ph1[:, 2048:2112]               # robot L1 out [128, 64] (slot A4)
    pr2 = ph2[:, 1024:1088]               # robot L2 out [64, 64] (slot B2)
    po = ph1[0:BL, 1536:1600]             # final out [64, 64] (slot A3)

    # chunk -> psum col offsets of its (1 or 2) 504-col slices; 2-slice
    # chunks always sit on adjacent +512 slots so one strided AP covers both
    CSL1 = [(0,), (512, 1024), (1536, 2048), (0, 512), (1024,)]   # L1 in ph1
    CSL2 = [(0,), (512, 1024), (0, 512), (512, 1024), (0,)]       # L2 in ph2

    def pin(ph, offs, cols):
        """PSUM input AP for a chunk's relu: 1 or 2 strided 504-col slices."""
        part = list(ph.ap[0])
        if len(offs) == 1:
            return bass.AP(ph.tensor, ph.offset + offs[0], [part, [1, cols]])
        assert offs[1] == offs[0] + 512
        return bass.AP(ph.tensor, ph.offset + offs[0], [part, [512, 2], [1, SL]])

    def h2c(c):
        return h2d[:, CSTART[c]:CSTART[c] + CB[c], :]

    # --- semaphores ---
    sdw = nc.alloc_semaphore("sdw")   # wp DMA
    sdh = nc.alloc_semaphore("sdh")   # hTb DMA
    sdf = nc.alloc_semaphore("sdf")   # fp DMA
    sp = nc.alloc_semaphore("sp")     # PE groups
    sa = nc.alloc_semaphore("sa")     # ACT ops
    sv = nc.alloc_semaphore("sv")     # DVE ops
    sg = nc.alloc_semaphore("sg")     # GPS ops
    sq = nc.alloc_semaphore("sq")     # out DMA (inc only)
    all_sems = [sdw, sdh, sdf, sp, sa, sv, sg, sq]

    with nc.Block(no_gpsimd_drain=True) as block:

        @block.sync
        def _(sync):
            sync.dma_start(out=wp, in_=d_wp).then_inc(sdw, 16)
            sync.dma_start(out=fp, in_=d_fp).then_inc(sdf, 16)
            sync.wait_ge(sv, 9)
            sync.dma_start(out=d_out, in_=out_sb).then_inc(sq, 16)

        @block.tensor
        def _(tensor):
            # sp: 1=rMM1 2=L1c0 3=L1c1 4=rMM2 5=L2c0 6=L1c2 7=L2c1 8=L1c3
            #     9=L2c2 10=L1c4 11=L2c3 12=L2c4 13=finals
            def lx(ph, csl, lhs, rhs, c, *waits):
                for s, v in waits:
                    tensor.wait_ge(s, v)
                last = None
                for o, t0 in zip(csl[c], range(CTS[c], CTS[c] + CT[c], SL)):
                    w = min(SL, CTS[c] + CT[c] - t0)
                    last = tensor.matmul(ph[:, o:o + w], lhs, rhs[:, t0:t0 + w],
                                         start=True, stop=True)
                last.then_inc(sp)

            def l1(c, *waits):
                lx(ph1, CSL1, wh1a, hTb, c, *waits)

            def l2(c, *waits):
                lx(ph2, CSL2, wh2, h1, c, *waits)

            # p-state warmup: dummy matmuls on garbage SBUF while DMAs land;
            # outputs go to chunk slots later reset by start=True matmuls
            for wo in (0, 512, 1024, 1536):
                tensor.matmul(ph1[:, wo:wo + SL], hTb[0:6, 0:128],
                              hTb[:, 504:1008], start=True, stop=True)
            tensor.wait_ge(sdw, 16)
            tensor.matmul(pr1, wr1a, rTa, start=True, stop=True).then_inc(sp)   # 1
            l1(0, (sdh, 16))                 # 2
            l1(1)                            # 3
            tensor.wait_ge(sa, 1)
            tensor.matmul(pr2, wr2, r1[:], start=True, stop=True).then_inc(sp)  # 4
            l2(0, (sa, 2))                   # 5
            l1(2)                            # 6  (A4 WAR sa>=1 implied)
            l2(1, (sa, 3))                   # 7
            l1(3)                            # 8  (A0/A1 WAR sa>=2/3 implied)
            l2(2, (sa, 6), (sv, 1))          # 9  (B0 WAR sa>=5, B1 WAR relu2c1)
            l1(4)                            # 10 (A2 WAR sa>=3 implied)
            l2(3, (sv, 2), (sa, 7))          # 11 (B1/B2 WAR relu2c2/relu2c1)
            l2(4, (sa, 8))                   # 12 (B0 WAR sv>=2 implied)
            tensor.wait_ge(sv, 8)            # all reductions done
            tensor.wait_ge(sg, 1)            # r2a ones row set
            tensor.matmul(po, c12[:], Wc, start=True, stop=False)
            tensor.matmul(po, r2a[:], Ara, start=False, stop=True).then_inc(sp)  # 13

        @block.scalar
        def _(scalar):
            # sa: 1=r1relu 2=relu1c0 3=relu1c1 4=r2relu 5=relu2c0 6=relu1c2
            #     7=relu1c3 8=relu1c4 9=relu2c4
            scalar.dma_start(out=hTb, in_=d_hTb).then_inc(sdh, 16)
            scalar.wait_ge(sp, 1)
            scalar.activation(r1[:], pr1, AF.Relu).then_inc(sa)
            scalar.wait_ge(sp, 2)
            scalar.activation(h1[:, 0:CT[0]], pin(ph1, CSL1[0], CT[0]),
                              AF.Relu).then_inc(sa)
            scalar.wait_ge(sp, 3)
            scalar.activation(h1[:, CTS[1]:CTS[2]], pin(ph1, CSL1[1], CT[1]),
                              AF.Relu).then_inc(sa)
            scalar.wait_ge(sp, 4)
            scalar.wait_ge(sdf, 16)
            scalar.activation(r2a[0:F, :], pr2, AF.Relu, bias=br2).then_inc(sa)
            scalar.wait_ge(sp, 5)
            scalar.activation(h2c(0), pin(ph2, CSL2[0], CT[0]), AF.Relu,
                              bias=bh2).then_inc(sa)
            scalar.wait_ge(sp, 6)
            scalar.activation(h1[:, CTS[2]:CTS[3]], pin(ph1, CSL1[2], CT[2]),
                              AF.Relu).then_inc(sa)
            scalar.wait_ge(sp, 8)
            scalar.activation(h1[:, CTS[3]:CTS[4]], pin(ph1, CSL1[3], CT[3]),
                              AF.Relu).then_inc(sa)
            scalar.wait_ge(sp, 10)
            scalar.activation(h1[:, CTS[4]:TOK], pin(ph1, CSL1[4], CT[4]),
                              AF.Relu).then_inc(sa)
            scalar.wait_ge(sp, 12)
            scalar.activation(h2c(4), pin(ph2, CSL2[4], CT[4]), AF.Relu,
                              bias=bh2).then_inc(sa)

        @block.vector
        def _(vector):
            # sv: 1=relu2c1 2=relu2c2 3=redc0 4=relu2c3 5=redc1 6=redc2
            #     7=redc3 8=redc4 9=copy
            def relu2(c, *waits):
                for s, v in waits:
                    vector.wait_ge(s, v)
                vector.tensor_scalar(h2c(c), pin(ph2, CSL2[c], CT[c]), bh2, 0.0,
                                     ALU.add, ALU.max).then_inc(sv)

            def red(c, in_, *waits):
                for s, v in waits:
                    vector.wait_ge(s, v)
                with nc.allow_low_precision("bf16 c12 is fine for the 2e-2 gate"):
                    vector.tensor_reduce(c12[:, CSTART[c]:CSTART[c] + CB[c]], in_,
                                         axis=AX.X, op=ALU.add).then_inc(sv)

            relu2(1, (sp, 7), (sdf, 16))
            relu2(2, (sp, 9))
            red(0, h2c(0), (sa, 5))
            relu2(3, (sp, 11))
            red(1, tmpb[:, 0:16, :], (sg, 3))
            red(2, tmpb[:, 16:32, :], (sg, 5))
            red(3, tmpb[:, 32:48, :], (sg, 7))
            red(4, h2c(4), (sa, 9))
            vector.wait_ge(sp, 13)
            vector.tensor_copy(out_sb[:], po).then_inc(sv)

        @block.gpsimd
        def _(gpsimd):
            # sg: 1=ones memset 2=ttAc1 3=ttBc1 4=ttAc2 5=ttBc2 6=ttAc3 7=ttBc3
            gpsimd.memset(r2a[F:F + 1, :], 1.0).then_inc(sg)

            def tt(rows, bs, *waits):
                for s, v in waits:
                    gpsimd.wait_ge(s, v)
                gpsimd.tensor_tensor(tmpa[:, rows, :], h2d[:, bs, 0:21],
                                     h2d[:, bs, 21:42], ALU.add).then_inc(sg)
                gpsimd.tensor_tensor(tmpb[:, rows, :], tmpa[:, rows, :],
                                     h2d[:, bs, 42:63], ALU.add).then_inc(sg)

            tt(slice(0, 16), slice(8, 24), (sv, 1))
            tt(slice(16, 32), slice(24, 40), (sv, 2))
            tt(slice(32, 48), slice(40, 56), (sv, 4))

    nc.clear_and_free_semaphores(all_sems)
    nc.compile()
    return nc


def _graph_vectors(edge_index):
    ei = np.asarray(edge_index)
    src, dst = ei[0].astype(np.int64), ei[1].astype(np.int64)
    f32 = np.float32
    deg = np.zeros(N, f32)
    np.add.at(deg, src, f32(1.0))
    dinv = np.where(deg > 0, deg.astype(f32) ** f32(-0.5), f32(0.0)).astype(f32)
    w = -(dinv[src] * dinv[dst])
    L = np.zeros((N, N), f32)
    np.add.at(L, (dst, src), w)
    v1 = L[0].astype(f32)
    v2 = (v1 @ L).astype(f32)
    return v1, v2


def _host_prep_const(v1, v2, robot_x, human_x, edge_index, wr1_w, wr1_b, wr2_w,
                     wr2_b, wh1_w, wh1_b, wh2_w, wh2_b, cheb_w, cheb_b):
    f32 = np.float32
    robot_x = np.ascontiguousarray(np.asarray(robot_x, f32))
    human_x = np.ascontiguousarray(np.asarray(human_x, f32))
    W0, W1, W2 = (np.asarray(cheb_w, f32)[k] for k in range(3))
    wh1_w = np.asarray(wh1_w, f32); wh1_b = np.asarray(wh1_b, f32)
    wh2_w = np.asarray(wh2_w, f32); wh2_b = np.asarray(wh2_b, f32)
    wr1_w = np.asarray(wr1_w, f32); wr1_b = np.asarray(wr1_b, f32)
    wr2_w = np.asarray(wr2_w, f32); wr2_b = np.asarray(wr2_b, f32)
    cheb_b = np.asarray(cheb_b, f32)

    alpha, beta = f32(v1[1]), f32(v2[1])
    Wc = alpha * W1 + f32(2.0) * beta * W2
    Ar = W0 - W2 + v1[0] * W1 + f32(2.0) * v2[0] * W2

    wp = np.zeros((HID, 576), f32)
    wp[0:HUMAN_DIM, 0:128] = wh1_w
    wp[HUMAN_DIM, 0:128] = wh1_b
    wp[:, 128:192] = wh2_w
    wp[0:ROBOT_DIM, 192:320] = wr1_w
    wp[ROBOT_DIM, 192:320] = wr1_b
    wp[:, 384:448] = wr2_w
    wp[0:F, 448:512] = Wc
    wp[0:F, 512:576] = Ar
    wp[F, 512:576] = cheb_b
    fpk = np.zeros((HID, 2), f32)
    fpk[0:F, 0] = wh2_b
    fpk[0:F, 1] = wr2_b

    in_maps = []
    ones_tok = np.ones((1, TOK), f32)
    for c in range(NCORES):
        bs = slice(c * BL, (c + 1) * BL)
        hT = human_x[bs].transpose(2, 0, 1).reshape(HUMAN_DIM, TOK)
        wpc = wp.copy()
        wpc[0:ROBOT_DIM, 320:384] = robot_x[bs, 0, :].T
        wpc[ROBOT_DIM, 320:384] = f32(1.0)
        in_maps.append({
            "hTb": _bf16(np.vstack([hT, ones_tok])),
            "wp": _bf16(wpc),
            "fp": fpk,
        })
    return in_maps


# --------------------------------------------------------------------------
# general-v fallback (previous implementation, fp32/f32r)
# --------------------------------------------------------------------------

def _build_bass_general():
    import os

    import concourse.bass as bass
    from concourse import bacc, mybir

    TOKG = BL * NH
    SLG = 504
    NPAIR = 4
    PBATCH = 16

    f32 = mybir.dt.float32
    f32r = mybir.dt.float32 if os.environ.get("DGCRNN_NO_F32R") else mybir.dt.float32r
    AF = mybir.ActivationFunctionType
    ALU = mybir.AluOpType
    AX = mybir.AxisListType

    nc = bacc.Bacc("TRN2", target_bir_lowering=False, debug=False)

    d_hTa = nc.dram_tensor("hTa", [HUMAN_DIM + 1, TOKG], f32r, kind="ExternalInput").ap()
    d_pa = nc.dram_tensor("pa", [HID, 257], f32, kind="ExternalInput").ap()
    d_pb = nc.dram_tensor("pb", [HID, 320], f32, kind="ExternalInput").ap()
    d_pr = nc.dram_tensor("pr", [HID, 256], f32r, kind="ExternalInput").ap()
    d_out = nc.dram_tensor("out", [BL, F], f32, kind="ExternalOutput").ap()

    hTa = nc.alloc_sbuf_tensor("hTa_sb", [HUMAN_DIM + 1, TOKG], f32r).ap()
    pa = nc.alloc_sbuf_tensor("pa_sb", [HID, 257], f32).ap()
    pb = nc.alloc_sbuf_tensor("pb_sb", [HID, 320], f32).ap()
    pr_ = nc.alloc_sbuf_tensor("pr_sb", [HID, 256], f32r).ap()
    h1 = nc.alloc_sbuf_tensor("h1_sb", [HID, TOKG], f32r).ap()
    h2d = nc.alloc_sbuf_tensor("h2d_sb", [2 * F, BL, NH], f32).ap()
    tmp = nc.alloc_sbuf_tensor("tmp_sb", [2 * F, BL, NH], f32).ap()
    c12 = nc.alloc_sbuf_tensor("c12_sb", [2 * F, BL], f32).ap()
    r1 = nc.alloc_sbuf_tensor("r1_sb", [HID, BL], f32).ap()
    r2 = nc.alloc_sbuf_tensor("r2_sb", [F, BL], f32).ap()
    out_sb = nc.alloc_sbuf_tensor("out_sb", [BL, F], f32).ap()

    wr2 = pa[:, 0:64]
    br2 = pa[0:64, 64:65]
    rTa = pa[0:ROBOT_DIM + 1, 65:129]
    wr1a = pa[0:ROBOT_DIM + 1, 129:257]
    W12 = pb[:, 0:64]
    v12 = pb[:, 64:127]
    bh2d = pb[:, 127:128]
    Ar = pb[0:64, 128:192]
    onesr = pb[0:1, 192:256]
    chebb = pb[0:1, 256:320]
    wh2d = pr_[:, 0:128]
    wh1a = pr_[0:HUMAN_DIM + 1, 128:256]

    ph1 = nc.alloc_psum_tensor("ph1", [HID, 2048], f32).ap()
    ph2 = nc.alloc_psum_tensor("ph2", [2 * F, 2048], f32).ap()
    pr1 = ph1[:, 0:BL]
    pr2 = ph2[:F, 0:BL]
    po = ph2[:BL, 0:F]

    v12_b = bass.AP(v12.tensor, v12.offset, [list(v12.ap[0]), [0, PBATCH], [1, NH]])

    sdh = [nc.alloc_semaphore(f"sdh{c}") for c in range(NPAIR)]
    sdr = nc.alloc_semaphore("sdr")
    sdw = nc.alloc_semaphore("sdw")
    sdf = nc.alloc_semaphore("sdf")
    sp = nc.alloc_semaphore("sp")
    sa = nc.alloc_semaphore("sa")
    sv = nc.alloc_semaphore("sv")
    sg = nc.alloc_semaphore("sg")
    sq = nc.alloc_semaphore("sq")
    all_sems = sdh + [sdr, sdw, sdf, sp, sa, sv, sg]

    PC = 1008

    def ping(p):
        return (p % 2) * 1024

    def pair_b(p):
        return slice(p * PBATCH, (p + 1) * PBATCH)

    def ph_in(ph, p):
        o = ping(p)
        return bass.AP(ph.tensor, ph.offset + o, [list(ph.ap[0]), [512, 2], [1, SLG]])

    with nc.Block(no_gpsimd_drain=True) as block:

        @block.sync
        def _(sync):
            for c in range(NPAIR):
                sync.dma_start(
                    out=hTa[:, c * PC: (c + 1) * PC],
                    in_=d_hTa[:, c * PC: (c + 1) * PC],
                ).then_inc(sdh[c], 16)
            sync.wait_ge(sv, 8)
            sync.dma_start(out=d_out[:], in_=out_sb[:]).then_inc(sq, 16)

        @block.tensor
        def _(tensor):
            def l1(p, *waits):
                for s, v in waits:
                    tensor.wait_ge(s, v)
                o = ping(p)
                tensor.matmul(ph1[:, o: o + SLG], wh1a, hTa[:, p * PC: p * PC + SLG],
                              start=True, stop=True)
                tensor.matmul(ph1[:, o + 512: o + 512 + SLG], wh1a,
                              hTa[:, p * PC + SLG: (p + 1) * PC],
                              start=True, stop=True).then_inc(sp)

            def l2(p, *waits):
                for s, v in waits:
                    tensor.wait_ge(s, v)
                o = ping(p)
                tensor.matmul(ph2[:, o: o + SLG], wh2d, h1[:, p * PC: p * PC + SLG],
                              start=True, stop=True)
                tensor.matmul(ph2[:, o + 512: o + 512 + SLG], wh2d,
                              h1[:, p * PC + SLG: (p + 1) * PC],
                              start=True, stop=True).then_inc(sp)

            tensor.wait_ge(sdr, 16)
            tensor.matmul(pr1, wr1a, rTa, start=True, stop=True).then_inc(sp)
            tensor.wait_ge(sa, 1)
            tensor.matmul(pr2, wr2, r1[:], start=True, stop=True).then_inc(sp)
            tensor.wait_ge(sdw, 16)
            l1(0, (sdh[0], 16), (sa, 1))
            l1(1, (sdh[1], 16))
            l2(0, (sa, 3))
            l1(2, (sdh[2], 16), (sa, 3))
            l2(1, (sa, 4))
            l1(3, (sdh[3], 16), (sa, 4))
            l2(2, (sa, 6))
            l2(3, (sa, 8))
            tensor.wait_ge(sv, 7)
            tensor.wait_ge(sa, 9)
            tensor.wait_ge(sdf, 16)
            tensor.matmul(po, c12[:], W12, start=True, stop=False)
            tensor.matmul(po, r2[:], Ar, start=False, stop=False)
            tensor.matmul(po, onesr, chebb, start=False, stop=True).then_inc(sp)

        @block.scalar
        def _(scalar):
            scalar.dma_start(out=pa[:], in_=d_pa[:]).then_inc(sdr, 16)
            scalar.dma_start(out=pr_[:], in_=d_pr[:]).then_inc(sdw, 16)
            scalar.dma_start(out=pb[:], in_=d_pb[:]).then_inc(sdf, 16)
            scalar.wait_ge(sp, 1)
            scalar.activation(r1[:], pr1, AF.Relu).then_inc(sa)
            scalar.wait_ge(sp, 2)
            scalar.activation(r2[:], pr2, AF.Relu, bias=br2).then_inc(sa)
            scalar.wait_ge(sp, 3)
            scalar.activation(h1[:, 0:PC], ph_in(ph1, 0), AF.Relu).then_inc(sa)
            scalar.wait_ge(sp, 4)
            scalar.activation(h1[:, PC: 2 * PC], ph_in(ph1, 1), AF.Relu).then_inc(sa)
            scalar.wait_ge(sp, 5)
            scalar.wait_ge(sdf, 16)
            scalar.activation(h2d[:, pair_b(0), :], ph_in(ph2, 0), AF.Relu,
                              bias=bh2d).then_inc(sa)
            scalar.wait_ge(sp, 6)
            scalar.activation(h1[:, 2 * PC: 3 * PC], ph_in(ph1, 2), AF.Relu).then_inc(sa)
            scalar.wait_ge(sp, 7)
            scalar.activation(h2d[:, pair_b(1), :], ph_in(ph2, 1), AF.Relu,
                              bias=bh2d).then_inc(sa)
            scalar.wait_ge(sp, 8)
            scalar.activation(h1[:, 3 * PC: 4 * PC], ph_in(ph1, 3), AF.Relu).then_inc(sa)
            scalar.wait_ge(sp, 9)
            scalar.activation(h2d[:, pair_b(2), :], ph_in(ph2, 2), AF.Relu,
                              bias=bh2d).then_inc(sa)
            scalar.wait_ge(sp, 10)
            scalar.activation(h2d[:, pair_b(3), :], ph_in(ph2, 3), AF.Relu,
                              bias=bh2d).then_inc(sa)

        @block.vector
        def _(vector):
            def red(p, *waits):
                for s, v in waits:
                    vector.wait_ge(s, v)
                vector.tensor_reduce(c12[:, pair_b(p)], tmp[:, pair_b(p), :],
                                     axis=AX.X, op=ALU.add).then_inc(sv)

            vector.wait_ge(sa, 5)
            vector.wait_ge(sdf, 16)
            vector.tensor_tensor(tmp[:, pair_b(0), :], h2d[:, pair_b(0), :],
                                 v12_b, ALU.mult).then_inc(sv)
            red(0, (sv, 1))
            vector.wait_ge(sa, 7)
            vector.tensor_tensor(tmp[:, pair_b(1), :], h2d[:, pair_b(1), :],
                                 v12_b, ALU.mult).then_inc(sv)
            red(1, (sv, 3))
            vector.wait_ge(sa, 10)
            vector.tensor_tensor(tmp[:, pair_b(3), :], h2d[:, pair_b(3), :],
                                 v12_b, ALU.mult).then_inc(sv)
            red(3, (sv, 5))
            red(2, (sg, 1))
            vector.wait_ge(sp, 11)
            vector.tensor_copy(out_sb[:], po).then_inc(sv)

        @block.gpsimd
        def _(gpsimd):
            gpsimd.wait_ge(sa, 9)
            gpsimd.tensor_tensor(tmp[:, pair_b(2), :], h2d[:, pair_b(2), :],
                                 v12_b, ALU.mult).then_inc(sg)

    nc.clear_and_free_semaphores(all_sems)
    nc.compile()
    return nc


def _host_prep_general(v1, v2, robot_x, human_x, edge_index, wr1_w, wr1_b, wr2_w,
                       wr2_b, wh1_w, wh1_b, wh2_w, wh2_b, cheb_w, cheb_b):
    f32 = np.float32
    robot_x = np.ascontiguousarray(np.asarray(robot_x, f32))
    human_x = np.ascontiguousarray(np.asarray(human_x, f32))
    W0, W1, W2 = (np.asarray(cheb_w, f32)[k] for k in range(3))
    wh1_w = np.asarray(wh1_w, f32); wh1_b = np.asarray(wh1_b, f32)
    wh2_w = np.asarray(wh2_w, f32); wh2_b = np.asarray(wh2_b, f32)
    wr1_w = np.asarray(wr1_w, f32); wr1_b = np.asarray(wr1_b, f32)
    wr2_w = np.asarray(wr2_w, f32); wr2_b = np.asarray(wr2_b, f32)
    cheb_b = np.asarray(cheb_b, f32)

    pa = np.zeros((HID, 257), f32)
    pa[:, 0:64] = wr2_w
    pa[0:64, 64] = wr2_b
    pa[0:ROBOT_DIM, 129:257] = wr1_w
    pa[ROBOT_DIM, 129:257] = wr1_b
    pb = np.zeros((HID, 320), f32)
    pb[0:64, 0:64] = W1
    pb[64:128, 0:64] = f32(2.0) * W2
    pb[0:64, 64:127] = np.tile(v1[1:], (F, 1))
    pb[64:128, 64:127] = np.tile(v2[1:], (F, 1))
    pb[0:64, 127] = wh2_b
    pb[64:128, 127] = wh2_b
    pb[0:64, 128:192] = W0 - W2 + v1[0] * W1 + f32(2.0) * v2[0] * W2
    pb[0, 192:256] = f32(1.0)
    pb[0, 256:320] = cheb_b
    pr = np.zeros((HID, 256), f32)
    pr[:, 0:128] = np.hstack([wh2_w, wh2_w])
    pr[0:HUMAN_DIM, 128:256] = wh1_w
    pr[HUMAN_DIM, 128:256] = wh1_b
    shared = {"pa": pa, "pb": pb, "pr": pr}

    in_maps = []
    ones_tok = np.ones((1, TOK), f32)
    for c in range(NCORES):
        bs = slice(c * BL, (c + 1) * BL)
        hT = human_x[bs].transpose(2, 0, 1).reshape(HUMAN_DIM, TOK)
        m = dict(shared)
        m["hTa"] = np.ascontiguousarray(np.vstack([hT, ones_tok]))
        pac = shared["pa"].copy()
        pac[0:ROBOT_DIM, 65:129] = robot_x[bs, 0, :].T
        pac[ROBOT_DIM, 65:129] = f32(1.0)
        m["pa"] = pac
        in_maps.append(m)
    return in_maps


# --------------------------------------------------------------------------
# driver
# --------------------------------------------------------------------------

def run(inputs, trace=False, tmpdir=None):
    """Run the Bass kernel on 8 cores. Returns (full_output, BassKernelResults)."""
    from concourse.bass_utils import run_bass_kernel_spmd

    v1, v2 = _graph_vectors(inputs["edge_index"])
    const_ok = (np.allclose(v1[1:], v1[1], rtol=1e-5, atol=1e-7)
                and np.allclose(v2[1:], v2[1], rtol=1e-5, atol=1e-7))

    key = "const" if const_ok else "general"
    if key not in _STATE:
        _STATE[key] = _build_bass_const() if const_ok else _build_bass_general()
    nc = _STATE[key]

    if const_ok:
        in_maps = _host_prep_const(v1, v2, **inputs)
    else:
        in_maps = _host_prep_general(v1, v2, **inputs)
    res = run_bass_kernel_spmd(
        nc, in_maps, list(range(NCORES)), trace=trace, tmpdir=tmpdir
    )
    out = np.concatenate([res.results[c]["out"] for c in range(NCORES)], axis=0)
    return np.ascontiguousarray(out, dtype=np.float32), res


def kernel(**inputs) -> np.ndarray:
    out, _ = run(inputs, trace=False)
    return out


# revision 18
# speedup vs baseline: 1.0680x; 1.0680x over previous
"""Trainium2 Bass kernel for nn_DGCRNN (ChebConv K=3 GNN, robot-node output).

Math: the reference returns only node 0 (robot) of the ChebConv output:
    out = r @ Ar + c1 @ W1 + c2 @ (2*W2) + cheb_b
with Ar = W0 - W2 + v1[0]*W1 + 2*v2[0]*W2, v1 = L_hat[0,:],
v2 = (L_hat @ L_hat)[0,:], c_k = sum_n v_k[n+1] * h_n over the 63 human-node
embeddings h_n, and r the robot embedding (both 2-layer relu MLPs).

Fast path (const-v): for the complete graph that setup_inputs() builds, every
human node has the same degree, so v1[1:] and v2[1:] are constant vectors.
Then c1 and c2 are both proportional to the plain segmented sum
S[f,b] = sum_n h2[f,b,n], and the entire v-weighting folds into one
host-computed combine matrix Wc = v1[1]*W1 + 2*v2[1]*W2:
    out = r @ Ar + S @ Wc + cheb_b
This removes the per-token v-multiply on device entirely. A general-v kernel
(the previous implementation) is kept as fallback for arbitrary graphs.

Sharding: pure data parallel over the batch dim (512 / 8 cores = 64 each).

Const-v implementation: bf16 datapath (fp32 PSUM accumulation, fp32 final
combine), 5 token chunks (8/16/16/16/8 batches) pipelined across engines:
PE does the MLP matmuls, ACT drains relu1 (+relu2 of the edge chunks), DVE
drains relu2 of the middle chunks and does the segmented reductions, GPSIMD
pre-folds 63->21 for two middle chunks so their reduce is cheap, SP issues
all DMAs (weights pack, fp32 pack, tokens, output).
"""

import numpy as np

B, N, F, HID = 512, 64, 64, 128
ROBOT_DIM, HUMAN_DIM = 9, 5
NCORES = 8
BL = B // NCORES      # 64 batches per core
NH = N - 1            # 63 human nodes
TOK = BL * NH         # 4032 human tokens per core

# const-v chunking: batch counts per chunk
CB = [8, 16, 16, 16, 8]
CSTART = [0, 8, 24, 40, 56]            # first batch of each chunk
CT = [c * NH for c in CB]              # tokens per chunk
CTS = [s * NH for s in CSTART]         # first token of each chunk
SL = 504                               # cols per matmul slice (<= 512 psum bank)

_STATE = {}


def _bf16(a):
    import ml_dtypes
    return np.asarray(a, np.float32).astype(ml_dtypes.bfloat16)


# --------------------------------------------------------------------------
# const-v kernel
# --------------------------------------------------------------------------

def _build_bass_const():
    import concourse.bass as bass
    from concourse import bacc, mybir

    f32 = mybir.dt.float32
    bf16 = mybir.dt.bfloat16
    AF = mybir.ActivationFunctionType
    ALU = mybir.AluOpType
    AX = mybir.AxisListType

    nc = bacc.Bacc("TRN2", target_bir_lowering=False, debug=False)

    # --- DRAM I/O (row-tight pieces so each DMA is small and lands early) ---
    d_hTb = nc.dram_tensor("hTb", [HUMAN_DIM + 1, TOK], bf16, kind="ExternalInput").ap()
    d_wpA = nc.dram_tensor("wpA", [ROBOT_DIM + 1, 320], bf16, kind="ExternalInput").ap()
    d_wpB = nc.dram_tensor("wpB", [HID, 128], bf16, kind="ExternalInput").ap()
    d_wpC = nc.dram_tensor("wpC", [F + 1, 128], bf16, kind="ExternalInput").ap()
    d_fp = nc.dram_tensor("fp", [F, 2], f32, kind="ExternalInput").ap()
    d_out = nc.dram_tensor("out", [BL, F], f32, kind="ExternalOutput").ap()

    # --- SBUF ---
    hTb = nc.alloc_sbuf_tensor("hTb_sb", [HUMAN_DIM + 1, TOK], bf16).ap()
    wpA = nc.alloc_sbuf_tensor("wpA_sb", [ROBOT_DIM + 1, 320], bf16).ap()
    wpB = nc.alloc_sbuf_tensor("wpB_sb", [HID, 128], bf16).ap()
    wpC = nc.alloc_sbuf_tensor("wpC_sb", [F + 1, 128], bf16).ap()
    fp = nc.alloc_sbuf_tensor("fp_sb", [F, 2], f32).ap()
    h1 = nc.alloc_sbuf_tensor("h1_sb", [HID, TOK], bf16).ap()
    h2d = nc.alloc_sbuf_tensor("h2d_sb", [F, BL, NH], bf16).ap()
    tmpa = nc.alloc_sbuf_tensor("tmpa_sb", [F, 48, 21], bf16).ap()
    tmpb = nc.alloc_sbuf_tensor("tmpb_sb", [F, 48, 21], bf16).ap()
    r1 = nc.alloc_sbuf_tensor("r1_sb", [HID, BL], bf16).ap()
    r2a = nc.alloc_sbuf_tensor("r2a_sb", [F + 1, BL], bf16).ap()
    c12 = nc.alloc_sbuf_tensor("c12_sb", [F, BL], bf16).ap()
    out_sb = nc.alloc_sbuf_tensor("out_sb", [BL, F], f32).ap()

    # pack slices (bf16)
    wh1a = wpA[0:HUMAN_DIM + 1, 0:128]    # L1 stationary [6, 128]
    wr1a = wpA[:, 128:256]                # robot L1 stationary [10, 128]
    rTa = wpA[:, 256:320]                 # robot tokens [10, 64]
    wh2 = wpB[:, 0:64]                    # L2 stationary [128, 64]
    wr2 = wpB[:, 64:128]                  # robot L2 stationary [128, 64]
    Wc = wpC[0:F, 0:64]                   # folded combine [64, 64]
    Ara = wpC[:, 64:128]                  # robot combine + cheb_b row [65, 64]
    bh2 = fp[:, 0:1]                      # wh2 bias [64, 1]
    br2 = fp[:, 1:2]                      # robot L2 bias [64, 1]

    # --- PSUM: ph1 and ph2 each have 4 512-col slots (4 banks) ---
    ph1 = nc.alloc_psum_tensor("ph1", [HID, 2048], f32).ap()
    ph2 = nc.alloc_psum_tensor("ph2", [F, 2048], f32).ap()
    pr1 = ph1[:, 1536:1600]               # robot L1 out (A3; L1c3 resets it)
    pr2 = ph2[:, 1536:1600]               # robot L2 out (B3; L2c3 resets it)
    po = ph2[0:BL, 1024:1088]             # final out (B2; drained by relu2c3)

    # chunk -> psum col offsets of its (1 or 2) 504-col slices; 2-slice
    # chunks always sit on adjacent +512 slots so one strided AP covers both
    CSL1 = [(0,), (512, 1024), (0, 512), (1024, 1536), (0,)]      # L1 in ph1
    CSL2 = [(0,), (512, 1024), (0, 512), (1024, 1536), (0,)]      # L2 in ph2

    def pin(ph, offs, cols):
        """PSUM input AP for a chunk's relu: 1 or 2 strided 504-col slices."""
        part = list(ph.ap[0])
        if len(offs) == 1:
            return bass.AP(ph.tensor, ph.offset + offs[0], [part, [1, cols]])
        assert offs[1] == offs[0] + 512
        return bass.AP(ph.tensor, ph.offset + offs[0], [part, [512, 2], [1, SL]])

    def h2c(c):
        return h2d[:, CSTART[c]:CSTART[c] + CB[c], :]

    # --- semaphores ---
    sdw = nc.alloc_semaphore("sdw")   # wpA DMA
    sdh0 = nc.alloc_semaphore("sdh0")  # hTb piece 0
    sdh1 = nc.alloc_semaphore("sdh1")  # hTb piece 1
    sdh2 = nc.alloc_semaphore("sdh2")  # hTb piece 2
    sdb = nc.alloc_semaphore("sdb")   # wpB DMA
    sdf = nc.alloc_semaphore("sdf")   # fp DMA
    sdc = nc.alloc_semaphore("sdc")   # wpC DMA
    sp = nc.alloc_semaphore("sp")     # PE groups
    sa = nc.alloc_semaphore("sa")     # ACT ops
    sv = nc.alloc_semaphore("sv")     # DVE ops
    sg = nc.alloc_semaphore("sg")     # GPS ops
    sq = nc.alloc_semaphore("sq")     # out DMA (inc only)
    all_sems = [sdw, sdh0, sdh1, sdh2, sdb, sdf, sdc, sp, sa, sv, sg, sq]

    with nc.Block(no_gpsimd_drain=True) as block:

        @block.sync
        def _(sync):
            sync.dma_start(out=wpA, in_=d_wpA).then_inc(sdw, 16)
            sync.dma_start(out=hTb[:, 0:504], in_=d_hTb[:, 0:504]).then_inc(sdh0, 16)
            sync.dma_start(out=hTb[:, 504:1512],
                           in_=d_hTb[:, 504:1512]).then_inc(sdh1, 16)
            sync.dma_start(out=wpB, in_=d_wpB).then_inc(sdb, 16)
            sync.dma_start(out=fp, in_=d_fp).then_inc(sdf, 16)
            sync.dma_start(out=hTb[:, 1512:TOK],
                           in_=d_hTb[:, 1512:TOK]).then_inc(sdh2, 16)
            sync.dma_start(out=wpC, in_=d_wpC).then_inc(sdc, 16)
            sync.wait_ge(sv, 9)
            sync.dma_start(out=d_out, in_=out_sb).then_inc(sq, 16)

        @block.tensor
        def _(tensor):
            # sp: 1=rMM1 2=L1c0 3=L1c1 4=rMM2 5=L2c0 6=L1c2 7=L2c1 8=L1c3
            #     9=L2c2 10=L1c4 11=L2c3 12=L2c4 13=finals
            def lx(ph, csl, lhs, rhs, c, *waits):
                for s, v in waits:
                    tensor.wait_ge(s, v)
                last = None
                for o, t0 in zip(csl[c], range(CTS[c], CTS[c] + CT[c], SL)):
                    w = min(SL, CTS[c] + CT[c] - t0)
                    last = tensor.matmul(ph[:, o:o + w], lhs, rhs[:, t0:t0 + w],
                                         start=True, stop=True)
                last.then_inc(sp)

            def l1(c, *waits):
                lx(ph1, CSL1, wh1a, hTb, c, *waits)

            def l2(c, *waits):
                lx(ph2, CSL2, wh2, h1, c, *waits)

            tensor.wait_ge(sdw, 16)
            tensor.matmul(pr1, wr1a, rTa, start=True, stop=True).then_inc(sp)   # 1
            l1(0, (sdh0, 16))                # 2
            l1(1, (sdh1, 16))                # 3
            tensor.wait_ge(sdb, 16)
            tensor.wait_ge(sa, 1)
            tensor.matmul(pr2, wr2, r1[:], start=True, stop=True).then_inc(sp)  # 4
            l2(0, (sa, 2))                   # 5
            l1(2, (sdh2, 16))                # 6  (A0/A1 WAR sa>=2/3 implied)
            l2(1, (sa, 3))                   # 7
            l1(3)                            # 8  (A2 WAR sa>=3, pr1 WAR sa>=1 implied)
            l2(2, (sa, 6), (sv, 1))          # 9  (B0 WAR sa>=5, B1 WAR relu2c1)
            l1(4)                            # 10 (A0 WAR sa>=6 implied)
            l2(3, (sa, 7))                   # 11 (B2/B3 WAR sv>=1/r2relu implied)
            l2(4, (sa, 8), (sv, 2))          # 12 (B0 WAR relu2c2)
            tensor.wait_ge(sv, 8)            # all reductions done
            tensor.wait_ge(sg, 1)            # r2a ones row set
            tensor.wait_ge(sdc, 16)
            tensor.matmul(po, c12[:], Wc, start=True, stop=False)
            tensor.matmul(po, r2a[:], Ara, start=False, stop=True).then_inc(sp)  # 13

        @block.scalar
        def _(scalar):
            # sa: 1=r1relu 2=relu1c0 3=relu1c1 4=r2relu 5=relu2c0 6=relu1c2
            #     7=relu1c3 8=relu1c4 9=relu2c4
            scalar.wait_ge(sp, 1)
            scalar.activation(r1[:], pr1, AF.Relu).then_inc(sa)
            scalar.wait_ge(sp, 2)
            scalar.activation(h1[:, 0:CT[0]], pin(ph1, CSL1[0], CT[0]),
                              AF.Relu).then_inc(sa)
            scalar.wait_ge(sp, 3)
            scalar.activation(h1[:, CTS[1]:CTS[2]], pin(ph1, CSL1[1], CT[1]),
                              AF.Relu).then_inc(sa)
            scalar.wait_ge(sp, 4)
            scalar.wait_ge(sdf, 16)
            scalar.activation(r2a[0:F, :], pr2, AF.Relu, bias=br2).then_inc(sa)
            scalar.wait_ge(sp, 5)
            scalar.activation(h2c(0), pin(ph2, CSL2[0], CT[0]), AF.Relu,
                              bias=bh2).then_inc(sa)
            scalar.wait_ge(sp, 6)
            scalar.activation(h1[:, CTS[2]:CTS[3]], pin(ph1, CSL1[2], CT[2]),
                              AF.Relu).then_inc(sa)
            scalar.wait_ge(sp, 8)
            scalar.activation(h1[:, CTS[3]:CTS[4]], pin(ph1, CSL1[3], CT[3]),
                              AF.Relu).then_inc(sa)
            scalar.wait_ge(sp, 10)
            scalar.activation(h1[:, CTS[4]:TOK], pin(ph1, CSL1[4], CT[4]),
                              AF.Relu).then_inc(sa)
            scalar.wait_ge(sp, 12)
            scalar.activation(h2c(4), pin(ph2, CSL2[4], CT[4]), AF.Relu,
                              bias=bh2).then_inc(sa)

        @block.vector
        def _(vector):
            # sv: 1=relu2c1 2=relu2c2 3=redc0 4=relu2c3 5=redc1 6=redc2
            #     7=redc3 8=redc4 9=copy
            def relu2(c, *waits):
                for s, v in waits:
                    vector.wait_ge(s, v)
                vector.tensor_scalar(h2c(c), pin(ph2, CSL2[c], CT[c]), bh2, 0.0,
                                     ALU.add, ALU.max).then_inc(sv)

            def red(c, in_, *waits):
                for s, v in waits:
                    vector.wait_ge(s, v)
                with nc.allow_low_precision("bf16 c12 is fine for the 2e-2 gate"):
                    vector.tensor_reduce(c12[:, CSTART[c]:CSTART[c] + CB[c]], in_,
                                         axis=AX.X, op=ALU.add).then_inc(sv)

            relu2(1, (sp, 7), (sdf, 16))
            relu2(2, (sp, 9))
            red(0, h2c(0), (sa, 5))
            relu2(3, (sp, 11))
            red(1, tmpb[:, 0:16, :], (sg, 3))
            red(2, tmpb[:, 16:32, :], (sg, 5))
            red(3, tmpb[:, 32:48, :], (sg, 7))
            red(4, h2c(4), (sa, 9))
            vector.wait_ge(sp, 13)
            vector.tensor_copy(out_sb[:], po).then_inc(sv)

        @block.gpsimd
        def _(gpsimd):
            # sg: 1=ones memset 2=ttAc1 3=ttBc1 4=ttAc2 5=ttBc2 6=ttAc3 7=ttBc3
            gpsimd.memset(r2a[F:F + 1, :], 1.0).then_inc(sg)

            def tt(rows, bs, *waits):
                for s, v in waits:
                    gpsimd.wait_ge(s, v)
                gpsimd.tensor_tensor(tmpa[:, rows, :], h2d[:, bs, 0:21],
                                     h2d[:, bs, 21:42], ALU.add).then_inc(sg)
                gpsimd.tensor_tensor(tmpb[:, rows, :], tmpa[:, rows, :],
                                     h2d[:, bs, 42:63], ALU.add).then_inc(sg)

            tt(slice(0, 16), slice(8, 24), (sv, 1))
            tt(slice(16, 32), slice(24, 40), (sv, 2))
            tt(slice(32, 48), slice(40, 56), (sv, 4))

    nc.clear_and_free_semaphores(all_sems)
    nc.compile()
    return nc


def _graph_vectors(edge_index):
    ei = np.asarray(edge_index)
    src, dst = ei[0].astype(np.int64), ei[1].astype(np.int64)
    f32 = np.float32
    deg = np.zeros(N, f32)
    np.add.at(deg, src, f32(1.0))
    dinv = np.where(deg > 0, deg.astype(f32) ** f32(-0.5), f32(0.0)).astype(f32)
    w = -(dinv[src] * dinv[dst])
    L = np.zeros((N, N), f32)
    np.add.at(L, (dst, src), w)
    v1 = L[0].astype(f32)
    v2 = (v1 @ L).astype(f32)
    return v1, v2


def _host_prep_const(v1, v2, robot_x, human_x, edge_index, wr1_w, wr1_b, wr2_w,
                     wr2_b, wh1_w, wh1_b, wh2_w, wh2_b, cheb_w, cheb_b):
    f32 = np.float32
    robot_x = np.ascontiguousarray(np.asarray(robot_x, f32))
    human_x = np.ascontiguousarray(np.asarray(human_x, f32))
    W0, W1, W2 = (np.asarray(cheb_w, f32)[k] for k in range(3))
    wh1_w = np.asarray(wh1_w, f32); wh1_b = np.asarray(wh1_b, f32)
    wh2_w = np.asarray(wh2_w, f32); wh2_b = np.asarray(wh2_b, f32)
    wr1_w = np.asarray(wr1_w, f32); wr1_b = np.asarray(wr1_b, f32)
    wr2_w = np.asarray(wr2_w, f32); wr2_b = np.asarray(wr2_b, f32)
    cheb_b = np.asarray(cheb_b, f32)

    alpha, beta = f32(v1[1]), f32(v2[1])
    Wc = alpha * W1 + f32(2.0) * beta * W2
    Ar = W0 - W2 + v1[0] * W1 + f32(2.0) * v2[0] * W2

    wpA = np.zeros((ROBOT_DIM + 1, 320), f32)
    wpA[0:HUMAN_DIM, 0:128] = wh1_w
    wpA[HUMAN_DIM, 0:128] = wh1_b
    wpA[0:ROBOT_DIM, 128:256] = wr1_w
    wpA[ROBOT_DIM, 128:256] = wr1_b
    wpB = np.zeros((HID, 128), f32)
    wpB[:, 0:64] = wh2_w
    wpB[:, 64:128] = wr2_w
    wpC = np.zeros((F + 1, 128), f32)
    wpC[0:F, 0:64] = Wc
    wpC[0:F, 64:128] = Ar
    wpC[F, 64:128] = cheb_b
    wpB16, wpC16 = _bf16(wpB), _bf16(wpC)
    fpk = np.zeros((F, 2), f32)
    fpk[:, 0] = wh2_b
    fpk[:, 1] = wr2_b

    in_maps = []
    ones_tok = np.ones((1, TOK), f32)
    for c in range(NCORES):
        bs = slice(c * BL, (c + 1) * BL)
        hT = human_x[bs].transpose(2, 0, 1).reshape(HUMAN_DIM, TOK)
        wpAc = wpA.copy()
        wpAc[0:ROBOT_DIM, 256:320] = robot_x[bs, 0, :].T
        wpAc[ROBOT_DIM, 256:320] = f32(1.0)
        in_maps.append({
            "hTb": _bf16(np.vstack([hT, ones_tok])),
            "wpA": _bf16(wpAc),
            "wpB": wpB16,
            "wpC": wpC16,
            "fp": fpk,
        })
    return in_maps


# --------------------------------------------------------------------------
# general-v fallback (previous implementation, fp32/f32r)
# --------------------------------------------------------------------------

def _build_bass_general():
    import os

    import concourse.bass as bass
    from concourse import bacc, mybir

    TOKG = BL * NH
    SLG = 504
    NPAIR = 4
    PBATCH = 16

    f32 = mybir.dt.float32
    f32r = mybir.dt.float32 if os.environ.get("DGCRNN_NO_F32R") else mybir.dt.float32r
    AF = mybir.ActivationFunctionType
    ALU = mybir.AluOpType
    AX = mybir.AxisListType

    nc = bacc.Bacc("TRN2", target_bir_lowering=False, debug=False)

    d_hTa = nc.dram_tensor("hTa", [HUMAN_DIM + 1, TOKG], f32r, kind="ExternalInput").ap()
    d_pa = nc.dram_tensor("pa", [HID, 257], f32, kind="ExternalInput").ap()
    d_pb = nc.dram_tensor("pb", [HID, 320], f32, kind="ExternalInput").ap()
    d_pr = nc.dram_tensor("pr", [HID, 256], f32r, kind="ExternalInput").ap()
    d_out = nc.dram_tensor("out", [BL, F], f32, kind="ExternalOutput").ap()

    hTa = nc.alloc_sbuf_tensor("hTa_sb", [HUMAN_DIM + 1, TOKG], f32r).ap()
    pa = nc.alloc_sbuf_tensor("pa_sb", [HID, 257], f32).ap()
    pb = nc.alloc_sbuf_tensor("pb_sb", [HID, 320], f32).ap()
    pr_ = nc.alloc_sbuf_tensor("pr_sb", [HID, 256], f32r).ap()
    h1 = nc.alloc_sbuf_tensor("h1_sb", [HID, TOKG], f32r).ap()
    h2d = nc.alloc_sbuf_tensor("h2d_sb", [2 * F, BL, NH], f32).ap()
    tmp = nc.alloc_sbuf_tensor("tmp_sb", [2 * F, BL, NH], f32).ap()
    c12 = nc.alloc_sbuf_tensor("c12_sb", [2 * F, BL], f32).ap()
    r1 = nc.alloc_sbuf_tensor("r1_sb", [HID, BL], f32).ap()
    r2 = nc.alloc_sbuf_tensor("r2_sb", [F, BL], f32).ap()
    out_sb = nc.alloc_sbuf_tensor("out_sb", [BL, F], f32).ap()

    wr2 = pa[:, 0:64]
    br2 = pa[0:64, 64:65]
    rTa = pa[0:ROBOT_DIM + 1, 65:129]
    wr1a = pa[0:ROBOT_DIM + 1, 129:257]
    W12 = pb[:, 0:64]
    v12 = pb[:, 64:127]
    bh2d = pb[:, 127:128]
    Ar = pb[0:64, 128:192]
    onesr = pb[0:1, 192:256]
    chebb = pb[0:1, 256:320]
    wh2d = pr_[:, 0:128]
    wh1a = pr_[0:HUMAN_DIM + 1, 128:256]

    ph1 = nc.alloc_psum_tensor("ph1", [HID, 2048], f32).ap()
    ph2 = nc.alloc_psum_tensor("ph2", [2 * F, 2048], f32).ap()
    pr1 = ph1[:, 0:BL]
    pr2 = ph2[:F, 0:BL]
    po = ph2[:BL, 0:F]

    v12_b = bass.AP(v12.tensor, v12.offset, [list(v12.ap[0]), [0, PBATCH], [1, NH]])

    sdh = [nc.alloc_semaphore(f"sdh{c}") for c in range(NPAIR)]
    sdr = nc.alloc_semaphore("sdr")
    sdw = nc.alloc_semaphore("sdw")
    sdf = nc.alloc_semaphore("sdf")
    sp = nc.alloc_semaphore("sp")
    sa = nc.alloc_semaphore("sa")
    sv = nc.alloc_semaphore("sv")
    sg = nc.alloc_semaphore("sg")
    sq = nc.alloc_semaphore("sq")
    all_sems = sdh + [sdr, sdw, sdf, sp, sa, sv, sg]

    PC = 1008

    def ping(p):
        return (p % 2) * 1024

    def pair_b(p):
        return slice(p * PBATCH, (p + 1) * PBATCH)

    def ph_in(ph, p):
        o = ping(p)
        return bass.AP(ph.tensor, ph.offset + o, [list(ph.ap[0]), [512, 2], [1, SLG]])

    with nc.Block(no_gpsimd_drain=True) as block:

        @block.sync
        def _(sync):
            for c in range(NPAIR):
                sync.dma_start(
                    out=hTa[:, c * PC: (c + 1) * PC],
                    in_=d_hTa[:, c * PC: (c + 1) * PC],
                ).then_inc(sdh[c], 16)
            sync.wait_ge(sv, 8)
            sync.dma_start(out=d_out[:], in_=out_sb[:]).then_inc(sq, 16)

        @block.tensor
        def _(tensor):
            def l1(p, *waits):
                for s, v in waits:
                    tensor.wait_ge(s, v)
                o = ping(p)
                tensor.matmul(ph1[:, o: o + SLG], wh1a, hTa[:, p * PC: p * PC + SLG],
                              start=True, stop=True)
                tensor.matmul(ph1[:, o + 512: o + 512 + SLG], wh1a,
                              hTa[:, p * PC + SLG: (p + 1) * PC],
                              start=True, stop=True).then_inc(sp)

            def l2(p, *waits):
                for s, v in waits:
                    tensor.wait_ge(s, v)
                o = ping(p)
                tensor.matmul(ph2[:, o: o + SLG], wh2d, h1[:, p * PC: p * PC + SLG],
                              start=True, stop=True)
                tensor.matmul(ph2[:, o + 512: o + 512 + SLG], wh2d,
                              h1[:, p * PC + SLG: (p + 1) * PC],
                              start=True, stop=True).then_inc(sp)

            tensor.wait_ge(sdr, 16)
            tensor.matmul(pr1, wr1a, rTa, start=True, stop=True).then_inc(sp)
            tensor.wait_ge(sa, 1)
            tensor.matmul(pr2, wr2, r1[:], start=True, stop=True).then_inc(sp)
            tensor.wait_ge(sdw, 16)
            l1(0, (sdh[0], 16), (sa, 1))
            l1(1, (sdh[1], 16))
            l2(0, (sa, 3))
            l1(2, (sdh[2], 16), (sa, 3))
            l2(1, (sa, 4))
            l1(3, (sdh[3], 16), (sa, 4))
            l2(2, (sa, 6))
            l2(3, (sa, 8))
            tensor.wait_ge(sv, 7)
            tensor.wait_ge(sa, 9)
            tensor.wait_ge(sdf, 16)
            tensor.matmul(po, c12[:], W12, start=True, stop=False)
            tensor.matmul(po, r2[:], Ar, start=False, stop=False)
            tensor.matmul(po, onesr, chebb, start=False, stop=True).then_inc(sp)

        @block.scalar
        def _(scalar):
            scalar.dma_start(out=pa[:], in_=d_pa[:]).then_inc(sdr, 16)
            scalar.dma_start(out=pr_[:], in_=d_pr[:]).then_inc(sdw, 16)
            scalar.dma_start(out=pb[:], in_=d_pb[:]).then_inc(sdf, 16)
            scalar.wait_ge(sp, 1)
            scalar.activation(r1[:], pr1, AF.Relu).then_inc(sa)
            scalar.wait_ge(sp, 2)
            scalar.activation(r2[:], pr2, AF.Relu, bias=br2).then_inc(sa)
            scalar.wait_ge(sp, 3)
            scalar.activation(h1[:, 0:PC], ph_in(ph1, 0), AF.Relu).then_inc(sa)
            scalar.wait_ge(sp, 4)
            scalar.activation(h1[:, PC: 2 * PC], ph_in(ph1, 1), AF.Relu).then_inc(sa)
            scalar.wait_ge(sp, 5)
            scalar.wait_ge(sdf, 16)
            scalar.activation(h2d[:, pair_b(0), :], ph_in(ph2, 0), AF.Relu,
                              bias=bh2d).then_inc(sa)
            scalar.wait_ge(sp, 6)
            scalar.activation(h1[:, 2 * PC: 3 * PC], ph_in(ph1, 2), AF.Relu).then_inc(sa)
            scalar.wait_ge(sp, 7)
            scalar.activation(h2d[:, pair_b(1), :], ph_in(ph2, 1), AF.Relu,
                              bias=bh2d).then_inc(sa)
            scalar.wait_ge(sp, 8)
            scalar.activation(h1[:, 3 * PC: 4 * PC], ph_in(ph1, 3), AF.Relu).then_inc(sa)
            scalar.wait_ge(sp, 9)
            scalar.activation(h2d[:, pair_b(2), :], ph_in(ph2, 2), AF.Relu,
                              bias=bh2d).then_inc(sa)
            scalar.wait_ge(sp, 10)
            scalar.activation(h2d[:, pair_b(3), :], ph_in(ph2, 3), AF.Relu,
                              bias=bh2d).then_inc(sa)

        @block.vector
        def _(vector):
            def red(p, *waits):
                for s, v in waits:
                    vector.wait_ge(s, v)
                vector.tensor_reduce(c12[:, pair_b(p)], tmp[:, pair_b(p), :],
                                     axis=AX.X, op=ALU.add).then_inc(sv)

            vector.wait_ge(sa, 5)
            vector.wait_ge(sdf, 16)
            vector.tensor_tensor(tmp[:, pair_b(0), :], h2d[:, pair_b(0), :],
                                 v12_b, ALU.mult).then_inc(sv)
            red(0, (sv, 1))
            vector.wait_ge(sa, 7)
            vector.tensor_tensor(tmp[:, pair_b(1), :], h2d[:, pair_b(1), :],
                                 v12_b, ALU.mult).then_inc(sv)
            red(1, (sv, 3))
            vector.wait_ge(sa, 10)
            vector.tensor_tensor(tmp[:, pair_b(3), :], h2d[:, pair_b(3), :],
                                 v12_b, ALU.mult).then_inc(sv)
            red(3, (sv, 5))
            red(2, (sg, 1))
            vector.wait_ge(sp, 11)
            vector.tensor_copy(out_sb[:], po).then_inc(sv)

        @block.gpsimd
        def _(gpsimd):
            gpsimd.wait_ge(sa, 9)
            gpsimd.tensor_tensor(tmp[:, pair_b(2), :], h2d[:, pair_b(2), :],
                                 v12_b, ALU.mult).then_inc(sg)

    nc.clear_and_free_semaphores(all_sems)
    nc.compile()
    return nc


def _host_prep_general(v1, v2, robot_x, human_x, edge_index, wr1_w, wr1_b, wr2_w,
                       wr2_b, wh1_w, wh1_b, wh2_w, wh2_b, cheb_w, cheb_b):
    f32 = np.float32
    robot_x = np.ascontiguousarray(np.asarray(robot_x, f32))
    human_x = np.ascontiguousarray(np.asarray(human_x, f32))
    W0, W1, W2 = (np.asarray(cheb_w, f32)[k] for k in range(3))
    wh1_w = np.asarray(wh1_w, f32); wh1_b = np.asarray(wh1_b, f32)
    wh2_w = np.asarray(wh2_w, f32); wh2_b = np.asarray(wh2_b, f32)
    wr1_w = np.asarray(wr1_w, f32); wr1_b = np.asarray(wr1_b, f32)
    wr2_w = np.asarray(wr2_w, f32); wr2_b = np.asarray(wr2_b, f32)
    cheb_b = np.asarray(cheb_b, f32)

    pa = np.zeros((HID, 257), f32)
    pa[:, 0:64] = wr2_w
    pa[0:64, 64] = wr2_b
    pa[0:ROBOT_DIM, 129:257] = wr1_w
    pa[ROBOT_DIM, 129:257] = wr1_b
    pb = np.zeros((HID, 320), f32)
    pb[0:64, 0:64] = W1
    pb[64:128, 0:64] = f32(2.0) * W2
    pb[0:64, 64:127] = np.tile(v1[1:], (F, 1))
    pb[64:128, 64:127] = np.tile(v2[1:], (F, 1))
    pb[0:64, 127] = wh2_b
    pb[64:128, 127] = wh2_b
    pb[0:64, 128:192] = W0 - W2 + v1[0] * W1 + f32(2.0) * v2[0] * W2
    pb[0, 192:256] = f32(1.0)
    pb[0, 256:320] = cheb_b
    pr = np.zeros((HID, 256), f32)
    pr[:, 0:128] = np.hstack([wh2_w, wh2_w])
    pr[0:HUMAN_DIM, 128:256] = wh1_w
    pr[HUMAN_DIM, 128:256] = wh1_b
    shared = {"pa": pa, "pb": pb, "pr": pr}

    in_maps = []
    ones_tok = np.ones((1, TOK), f32)
    for c in range(NCORES):
        bs = slice(c * BL, (c + 1) * BL)
        hT = human_x[bs].transpose(2, 0, 1).reshape(HUMAN_DIM, TOK)
        m = dict(shared)
        m["hTa"] = np.ascontiguousarray(np.vstack([hT, ones_tok]))
        pac = shared["pa"].copy()
        pac[0:ROBOT_DIM, 65:129] = robot_x[bs, 0, :].T
        pac[ROBOT_DIM, 65:129] = f32(1.0)
        m["pa"] = pac
        in_maps.append(m)
    return in_maps


# --------------------------------------------------------------------------
# driver
# --------------------------------------------------------------------------

def run(inputs, trace=False, tmpdir=None):
    """Run the Bass kernel on 8 cores. Returns (full_output, BassKernelResults)."""
    from concourse.bass_utils import run_bass_kernel_spmd

    v1, v2 = _graph_vectors(inputs["edge_index"])
    const_ok = (np.allclose(v1[1:], v1[1], rtol=1e-5, atol=1e-7)
                and np.allclose(v2[1:], v2[1], rtol=1e-5, atol=1e-7))

    key = "const" if const_ok else "general"
    if key not in _STATE:
        _STATE[key] = _build_bass_const() if const_ok else _build_bass_general()
    nc = _STATE[key]

    if const_ok:
        in_maps = _host_prep_const(v1, v2, **inputs)
    else:
        in_maps = _host_prep_general(v1, v2, **inputs)
    res = run_bass_kernel_spmd(
        nc, in_maps, list(range(NCORES)), trace=trace, tmpdir=tmpdir
    )
    out = np.concatenate([res.results[c]["out"] for c in range(NCORES)], axis=0)
    return np.ascontiguousarray(out, dtype=np.float32), res


def kernel(**inputs) -> np.ndarray:
    out, _ = run(inputs, trace=False)
    return out


# revision 28
# speedup vs baseline: 1.0805x; 1.0117x over previous
"""Trainium2 Bass kernel for nn_DGCRNN (ChebConv K=3 GNN, robot-node output).

Math: the reference returns only node 0 (robot) of the ChebConv output:
    out = r @ Ar + c1 @ W1 + c2 @ (2*W2) + cheb_b
with Ar = W0 - W2 + v1[0]*W1 + 2*v2[0]*W2, v1 = L_hat[0,:],
v2 = (L_hat @ L_hat)[0,:], c_k = sum_n v_k[n+1] * h_n over the 63 human-node
embeddings h_n, and r the robot embedding (both 2-layer relu MLPs).

Fast path (const-v): for the complete graph that setup_inputs() builds, every
human node has the same degree, so v1[1:] and v2[1:] are constant vectors.
Then c1 and c2 are both proportional to the plain segmented sum
S[f,b] = sum_n h2[f,b,n], and the entire v-weighting folds into one
host-computed combine matrix Wc = v1[1]*W1 + 2*v2[1]*W2:
    out = r @ Ar + S @ Wc + cheb_b
This removes the per-token v-multiply on device entirely. A general-v kernel
(the previous implementation) is kept as fallback for arbitrary graphs.

Sharding: pure data parallel over the batch dim (512 / 8 cores = 64 each).

Const-v implementation: bf16 datapath (fp32 PSUM accumulation, fp32 final
combine), 5 token chunks (8/16/16/16/8 batches) pipelined across engines:
PE does the MLP matmuls, ACT drains relu1 (+relu2 of the edge chunks), DVE
drains relu2 of the middle chunks and does the segmented reductions, GPSIMD
pre-folds 63->21 for two middle chunks so their reduce is cheap, SP issues
all DMAs (weights pack, fp32 pack, tokens, output).
"""

import numpy as np

B, N, F, HID = 512, 64, 64, 128
ROBOT_DIM, HUMAN_DIM = 9, 5
NCORES = 8
BL = B // NCORES      # 64 batches per core
NH = N - 1            # 63 human nodes
TOK = BL * NH         # 4032 human tokens per core

# const-v chunking: batch counts per chunk
CB = [8, 16, 16, 16, 8]
CSTART = [0, 8, 24, 40, 56]            # first batch of each chunk
CT = [c * NH for c in CB]              # tokens per chunk
CTS = [s * NH for s in CSTART]         # first token of each chunk
SL = 504                               # cols per matmul slice (<= 512 psum bank)

_STATE = {}


def _bf16(a):
    import ml_dtypes
    return np.asarray(a, np.float32).astype(ml_dtypes.bfloat16)


# --------------------------------------------------------------------------
# const-v kernel
# --------------------------------------------------------------------------

def _build_bass_const():
    import concourse.bass as bass
    from concourse import bacc, mybir

    f32 = mybir.dt.float32
    bf16 = mybir.dt.bfloat16
    AF = mybir.ActivationFunctionType
    ALU = mybir.AluOpType
    AX = mybir.AxisListType

    nc = bacc.Bacc("TRN2", target_bir_lowering=False, debug=False)

    # --- DRAM I/O (row-tight pieces so each DMA is small and lands early) ---
    d_hTb = nc.dram_tensor("hTb", [HUMAN_DIM + 1, TOK], bf16, kind="ExternalInput").ap()
    d_wpA = nc.dram_tensor("wpA", [ROBOT_DIM + 1, 320], bf16, kind="ExternalInput").ap()
    d_wpB = nc.dram_tensor("wpB", [HID, 128], bf16, kind="ExternalInput").ap()
    d_wpC = nc.dram_tensor("wpC", [F + 1, 128], bf16, kind="ExternalInput").ap()
    d_fp = nc.dram_tensor("fp", [F, 2], f32, kind="ExternalInput").ap()
    d_out = nc.dram_tensor("out", [BL, F], f32, kind="ExternalOutput").ap()

    # --- SBUF ---
    hTb = nc.alloc_sbuf_tensor("hTb_sb", [HUMAN_DIM + 1, TOK], bf16).ap()
    wpA = nc.alloc_sbuf_tensor("wpA_sb", [ROBOT_DIM + 1, 320], bf16).ap()
    wpB = nc.alloc_sbuf_tensor("wpB_sb", [HID, 128], bf16).ap()
    wpC = nc.alloc_sbuf_tensor("wpC_sb", [F + 1, 128], bf16).ap()
    fp = nc.alloc_sbuf_tensor("fp_sb", [F, 2], f32).ap()
    h1 = nc.alloc_sbuf_tensor("h1_sb", [HID, TOK], bf16).ap()
    h2d = nc.alloc_sbuf_tensor("h2d_sb", [F, BL, NH], bf16).ap()
    tmpa = nc.alloc_sbuf_tensor("tmpa_sb", [F, 48, 21], bf16).ap()
    tmpb = nc.alloc_sbuf_tensor("tmpb_sb", [F, 48, 21], bf16).ap()
    r1 = nc.alloc_sbuf_tensor("r1_sb", [HID, BL], bf16).ap()
    r2a = nc.alloc_sbuf_tensor("r2a_sb", [F + 1, BL], bf16).ap()
    c12 = nc.alloc_sbuf_tensor("c12_sb", [F, BL], bf16).ap()
    out_sb = nc.alloc_sbuf_tensor("out_sb", [BL, F], f32).ap()

    # pack slices (bf16)
    wh1a = wpA[0:HUMAN_DIM + 1, 0:128]    # L1 stationary [6, 128]
    wr1a = wpA[:, 128:256]                # robot L1 stationary [10, 128]
    rTa = wpA[:, 256:320]                 # robot tokens [10, 64]
    wh2 = wpB[:, 0:64]                    # L2 stationary [128, 64]
    wr2 = wpB[:, 64:128]                  # robot L2 stationary [128, 64]
    Wc = wpC[0:F, 0:64]                   # folded combine [64, 64]
    Ara = wpC[:, 64:128]                  # robot combine + cheb_b row [65, 64]
    bh2 = fp[:, 0:1]                      # wh2 bias [64, 1]
    br2 = fp[:, 1:2]                      # robot L2 bias [64, 1]

    # --- PSUM: ph1 and ph2 each have 4 512-col slots (4 banks) ---
    ph1 = nc.alloc_psum_tensor("ph1", [HID, 2048], f32).ap()
    ph2 = nc.alloc_psum_tensor("ph2", [F, 2048], f32).ap()
    pr1 = ph1[:, 1536:1600]               # robot L1 out (in A3; L1c3 resets it)
    pr2 = ph2[:, 1536:1600]               # robot L2 out (in B3; L2c3 resets it)
    po = ph2[0:BL, 1024:1088]             # final out (in B2; drained by relu2c3)

    # chunk -> psum col offsets of its (1 or 2) 504-col slices; 2-slice
    # chunks always sit on adjacent +512 slots so one strided AP covers both
    CSL1 = [(0,), (512, 1024), (0, 512), (1024, 1536), (0,)]      # L1 in ph1
    CSL2 = [(0,), (512, 1024), (0, 512), (1024, 1536), (0,)]      # L2 in ph2

    def pin(ph, offs, cols):
        """PSUM input AP for a chunk's relu: 1 or 2 strided 504-col slices."""
        part = list(ph.ap[0])
        if len(offs) == 1:
            return bass.AP(ph.tensor, ph.offset + offs[0], [part, [1, cols]])
        assert offs[1] == offs[0] + 512
        return bass.AP(ph.tensor, ph.offset + offs[0], [part, [512, 2], [1, SL]])

    def h2c(c):
        return h2d[:, CSTART[c]:CSTART[c] + CB[c], :]

    # --- semaphores ---
    sdw = nc.alloc_semaphore("sdw")   # wpA DMA
    sdh0 = nc.alloc_semaphore("sdh0")  # hTb piece 0
    sdh1 = nc.alloc_semaphore("sdh1")  # hTb piece 1
    sdh2 = nc.alloc_semaphore("sdh2")  # hTb piece 2
    sdb = nc.alloc_semaphore("sdb")   # wpB DMA
    sdf = nc.alloc_semaphore("sdf")   # fp DMA
    sdc = nc.alloc_semaphore("sdc")   # wpC DMA
    sp = nc.alloc_semaphore("sp")     # PE groups
    sa = nc.alloc_semaphore("sa")     # ACT ops
    sv = nc.alloc_semaphore("sv")     # DVE ops
    sg = nc.alloc_semaphore("sg")     # GPS ops
    sq = nc.alloc_semaphore("sq")     # out DMA (inc only)
    all_sems = [sdw, sdh0, sdh1, sdh2, sdb, sdf, sdc, sp, sa, sv, sg, sq]

    with nc.Block(no_gpsimd_drain=True) as block:

        @block.sync
        def _(sync):
            sync.dma_start(out=wpA, in_=d_wpA).then_inc(sdw, 16)
            sync.dma_start(out=hTb[:, 0:504], in_=d_hTb[:, 0:504]).then_inc(sdh0, 16)
            sync.dma_start(out=hTb[:, 504:1512],
                           in_=d_hTb[:, 504:1512]).then_inc(sdh1, 16)
            sync.dma_start(out=hTb[:, 1512:TOK],
                           in_=d_hTb[:, 1512:TOK]).then_inc(sdh2, 16)
            sync.wait_ge(sv, 8)
            sync.dma_start(out=d_out, in_=out_sb).then_inc(sq, 16)

        @block.tensor
        def _(tensor):
            # sp: 1=rMM1 2=L1c0 3=L1c1 4=rMM2 5=L2c0 6=L1c2 7=L2c1 8=L1c3
            #     9=L2c2 10=L1c4 11=L2c3 12=L2c4 13=finals
            def lx(ph, csl, lhs, rhs, c, *waits):
                for s, v in waits:
                    tensor.wait_ge(s, v)
                last = None
                for o, t0 in zip(csl[c], range(CTS[c], CTS[c] + CT[c], SL)):
                    w = min(SL, CTS[c] + CT[c] - t0)
                    last = tensor.matmul(ph[:, o:o + w], lhs, rhs[:, t0:t0 + w],
                                         start=True, stop=True)
                last.then_inc(sp)

            def l1(c, *waits):
                lx(ph1, CSL1, wh1a, hTb, c, *waits)

            def l2(c, *waits):
                lx(ph2, CSL2, wh2, h1, c, *waits)

            tensor.wait_ge(sdw, 16)
            tensor.matmul(pr1, wr1a, rTa, start=True, stop=True).then_inc(sp)   # 1
            l1(0, (sdh0, 16))                # 2
            l1(1, (sdh1, 16))                # 3
            tensor.wait_ge(sdb, 16)
            tensor.wait_ge(sa, 1)
            tensor.matmul(pr2, wr2, r1[:], start=True, stop=True).then_inc(sp)  # 4
            l2(0, (sa, 2))                   # 5
            l1(2, (sdh2, 16), (sa, 3))       # 6  (A0/A1 WAR relu1c0/relu1c1)
            l2(1, (sa, 3))                   # 7  (B1/B2 fresh)
            l1(3)                            # 8  (A2/A3 WAR sa>=3, pr1 sa>=1 implied)
            l2(2, (sa, 6), (sv, 1))          # 9  (B0 WAR sa>=5, B1 WAR relu2c1)
            l1(4)                            # 10 (A0 WAR sa>=6 implied)
            l2(3, (sa, 7))                   # 11 (B2/B3 WAR sv>=1/r2relu implied)
            l2(4, (sv, 2), (sa, 8))          # 12 (B0 WAR relu2c2)
            tensor.wait_ge(sv, 7)            # all reductions done
            tensor.wait_ge(sg, 1)            # r2a ones row set
            tensor.wait_ge(sdc, 16)
            tensor.matmul(po, c12[:], Wc, start=True, stop=False)
            tensor.matmul(po, r2a[:], Ara, start=False, stop=True).then_inc(sp)  # 13

        @block.scalar
        def _(scalar):
            # sa: 1=r1relu 2=relu1c0 3=relu1c1 4=r2relu 5=relu2c0 6=relu1c2
            #     7=relu1c3 8=relu1c4 9=relu2c3 10=relu2c4
            scalar.dma_start(out=wpB, in_=d_wpB).then_inc(sdb, 16)
            scalar.dma_start(out=fp, in_=d_fp).then_inc(sdf, 16)
            scalar.dma_start(out=wpC, in_=d_wpC).then_inc(sdc, 16)
            scalar.wait_ge(sp, 1)
            scalar.activation(r1[:], pr1, AF.Relu).then_inc(sa)
            scalar.wait_ge(sp, 2)
            scalar.activation(h1[:, 0:CT[0]], pin(ph1, CSL1[0], CT[0]),
                              AF.Relu).then_inc(sa)
            scalar.wait_ge(sp, 3)
            scalar.activation(h1[:, CTS[1]:CTS[2]], pin(ph1, CSL1[1], CT[1]),
                              AF.Relu).then_inc(sa)
            scalar.wait_ge(sp, 4)
            scalar.wait_ge(sdf, 16)
            scalar.activation(r2a[0:F, :], pr2, AF.Relu, bias=br2).then_inc(sa)
            scalar.wait_ge(sp, 5)
            scalar.activation(h2c(0), pin(ph2, CSL2[0], CT[0]), AF.Relu,
                              bias=bh2).then_inc(sa)
            scalar.wait_ge(sp, 6)
            scalar.activation(h1[:, CTS[2]:CTS[3]], pin(ph1, CSL1[2], CT[2]),
                              AF.Relu).then_inc(sa)
            scalar.wait_ge(sp, 8)
            scalar.activation(h1[:, CTS[3]:CTS[4]], pin(ph1, CSL1[3], CT[3]),
                              AF.Relu).then_inc(sa)
            scalar.wait_ge(sp, 10)
            scalar.activation(h1[:, CTS[4]:TOK], pin(ph1, CSL1[4], CT[4]),
                              AF.Relu).then_inc(sa)
            scalar.wait_ge(sp, 11)
            scalar.activation(h2c(3), pin(ph2, CSL2[3], CT[3]), AF.Relu,
                              bias=bh2).then_inc(sa)
            scalar.wait_ge(sp, 12)
            scalar.activation(h2c(4), pin(ph2, CSL2[4], CT[4]), AF.Relu,
                              bias=bh2).then_inc(sa)

        @block.vector
        def _(vector):
            # sv: 1=relu2c1 2=relu2c2 3=redc0 4=redc1 5=redc2 6=redc3 7=redc4
            #     8=copy
            def relu2(c, *waits):
                for s, v in waits:
                    vector.wait_ge(s, v)
                vector.tensor_scalar(h2c(c), pin(ph2, CSL2[c], CT[c]), bh2, 0.0,
                                     ALU.add, ALU.max).then_inc(sv)

            def red(c, in_, *waits):
                for s, v in waits:
                    vector.wait_ge(s, v)
                with nc.allow_low_precision("bf16 c12 is fine for the 2e-2 gate"):
                    vector.tensor_reduce(c12[:, CSTART[c]:CSTART[c] + CB[c]], in_,
                                         axis=AX.X, op=ALU.add).then_inc(sv)

            relu2(1, (sp, 7), (sdf, 16))
            relu2(2, (sp, 9))
            red(0, h2c(0), (sa, 5))
            red(1, tmpb[:, 0:16, :], (sg, 3))
            red(2, tmpb[:, 16:32, :], (sg, 5))
            red(3, h2c(3), (sa, 9))
            red(4, h2c(4), (sa, 10))
            vector.wait_ge(sp, 13)
            vector.tensor_copy(out_sb[:], po).then_inc(sv)

        @block.gpsimd
        def _(gpsimd):
            # sg: 1=ones memset 2=ttAc1 3=ttBc1 4=ttAc2 5=ttBc2
            gpsimd.memset(r2a[F:F + 1, :], 1.0).then_inc(sg)

            def tt(rows, bs, *waits):
                for s, v in waits:
                    gpsimd.wait_ge(s, v)
                gpsimd.tensor_tensor(tmpa[:, rows, :], h2d[:, bs, 0:21],
                                     h2d[:, bs, 21:42], ALU.add).then_inc(sg)
                gpsimd.tensor_tensor(tmpb[:, rows, :], tmpa[:, rows, :],
                                     h2d[:, bs, 42:63], ALU.add).then_inc(sg)

            tt(slice(0, 16), slice(8, 24), (sv, 1))
            tt(slice(16, 32), slice(24, 40), (sv, 2))

    nc.clear_and_free_semaphores(all_sems)
    nc.compile()
    return nc


def _graph_vectors(edge_index):
    ei = np.asarray(edge_index)
    src, dst = ei[0].astype(np.int64), ei[1].astype(np.int64)
    f32 = np.float32
    deg = np.zeros(N, f32)
    np.add.at(deg, src, f32(1.0))
    dinv = np.where(deg > 0, deg.astype(f32) ** f32(-0.5), f32(0.0)).astype(f32)
    w = -(dinv[src] * dinv[dst])
    L = np.zeros((N, N), f32)
    np.add.at(L, (dst, src), w)
    v1 = L[0].astype(f32)
    v2 = (v1 @ L).astype(f32)
    return v1, v2


def _host_prep_const(v1, v2, robot_x, human_x, edge_index, wr1_w, wr1_b, wr2_w,
                     wr2_b, wh1_w, wh1_b, wh2_w, wh2_b, cheb_w, cheb_b):
    f32 = np.float32
    robot_x = np.ascontiguousarray(np.asarray(robot_x, f32))
    human_x = np.ascontiguousarray(np.asarray(human_x, f32))
    W0, W1, W2 = (np.asarray(cheb_w, f32)[k] for k in range(3))
    wh1_w = np.asarray(wh1_w, f32); wh1_b = np.asarray(wh1_b, f32)
    wh2_w = np.asarray(wh2_w, f32); wh2_b = np.asarray(wh2_b, f32)
    wr1_w = np.asarray(wr1_w, f32); wr1_b = np.asarray(wr1_b, f32)
    wr2_w = np.asarray(wr2_w, f32); wr2_b = np.asarray(wr2_b, f32)
    cheb_b = np.asarray(cheb_b, f32)

    alpha, beta = f32(v1[1]), f32(v2[1])
    Wc = alpha * W1 + f32(2.0) * beta * W2
    Ar = W0 - W2 + v1[0] * W1 + f32(2.0) * v2[0] * W2

    wpA = np.zeros((ROBOT_DIM + 1, 320), f32)
    wpA[0:HUMAN_DIM, 0:128] = wh1_w
    wpA[HUMAN_DIM, 0:128] = wh1_b
    wpA[0:ROBOT_DIM, 128:256] = wr1_w
    wpA[ROBOT_DIM, 128:256] = wr1_b
    wpB = np.zeros((HID, 128), f32)
    wpB[:, 0:64] = wh2_w
    wpB[:, 64:128] = wr2_w
    wpC = np.zeros((F + 1, 128), f32)
    wpC[0:F, 0:64] = Wc
    wpC[0:F, 64:128] = Ar
    wpC[F, 64:128] = cheb_b
    wpB16, wpC16 = _bf16(wpB), _bf16(wpC)
    fpk = np.zeros((F, 2), f32)
    fpk[:, 0] = wh2_b
    fpk[:, 1] = wr2_b

    in_maps = []
    ones_tok = np.ones((1, TOK), f32)
    for c in range(NCORES):
        bs = slice(c * BL, (c + 1) * BL)
        hT = human_x[bs].transpose(2, 0, 1).reshape(HUMAN_DIM, TOK)
        wpAc = wpA.copy()
        wpAc[0:ROBOT_DIM, 256:320] = robot_x[bs, 0, :].T
        wpAc[ROBOT_DIM, 256:320] = f32(1.0)
        in_maps.append({
            "hTb": _bf16(np.vstack([hT, ones_tok])),
            "wpA": _bf16(wpAc),
            "wpB": wpB16,
            "wpC": wpC16,
            "fp": fpk,
        })
    return in_maps


# --------------------------------------------------------------------------
# general-v fallback (previous implementation, fp32/f32r)
# --------------------------------------------------------------------------

def _build_bass_general():
    import os

    import concourse.bass as bass
    from concourse import bacc, mybir

    TOKG = BL * NH
    SLG = 504
    NPAIR = 4
    PBATCH = 16

    f32 = mybir.dt.float32
    f32r = mybir.dt.float32 if os.environ.get("DGCRNN_NO_F32R") else mybir.dt.float32r
    AF = mybir.ActivationFunctionType
    ALU = mybir.AluOpType
    AX = mybir.AxisListType

    nc = bacc.Bacc("TRN2", target_bir_lowering=False, debug=False)

    d_hTa = nc.dram_tensor("hTa", [HUMAN_DIM + 1, TOKG], f32r, kind="ExternalInput").ap()
    d_pa = nc.dram_tensor("pa", [HID, 257], f32, kind="ExternalInput").ap()
    d_pb = nc.dram_tensor("pb", [HID, 320], f32, kind="ExternalInput").ap()
    d_pr = nc.dram_tensor("pr", [HID, 256], f32r, kind="ExternalInput").ap()
    d_out = nc.dram_tensor("out", [BL, F], f32, kind="ExternalOutput").ap()

    hTa = nc.alloc_sbuf_tensor("hTa_sb", [HUMAN_DIM + 1, TOKG], f32r).ap()
    pa = nc.alloc_sbuf_tensor("pa_sb", [HID, 257], f32).ap()
    pb = nc.alloc_sbuf_tensor("pb_sb", [HID, 320], f32).ap()
    pr_ = nc.alloc_sbuf_tensor("pr_sb", [HID, 256], f32r).ap()
    h1 = nc.alloc_sbuf_tensor("h1_sb", [HID, TOKG], f32r).ap()
    h2d = nc.alloc_sbuf_tensor("h2d_sb", [2 * F, BL, NH], f32).ap()
    tmp = nc.alloc_sbuf_tensor("tmp_sb", [2 * F, BL, NH], f32).ap()
    c12 = nc.alloc_sbuf_tensor("c12_sb", [2 * F, BL], f32).ap()
    r1 = nc.alloc_sbuf_tensor("r1_sb", [HID, BL], f32).ap()
    r2 = nc.alloc_sbuf_tensor("r2_sb", [F, BL], f32).ap()
    out_sb = nc.alloc_sbuf_tensor("out_sb", [BL, F], f32).ap()

    wr2 = pa[:, 0:64]
    br2 = pa[0:64, 64:65]
    rTa = pa[0:ROBOT_DIM + 1, 65:129]
    wr1a = pa[0:ROBOT_DIM + 1, 129:257]
    W12 = pb[:, 0:64]
    v12 = pb[:, 64:127]
    bh2d = pb[:, 127:128]
    Ar = pb[0:64, 128:192]
    onesr = pb[0:1, 192:256]
    chebb = pb[0:1, 256:320]
    wh2d = pr_[:, 0:128]
    wh1a = pr_[0:HUMAN_DIM + 1, 128:256]

    ph1 = nc.alloc_psum_tensor("ph1", [HID, 2048], f32).ap()
    ph2 = nc.alloc_psum_tensor("ph2", [2 * F, 2048], f32).ap()
    pr1 = ph1[:, 0:BL]
    pr2 = ph2[:F, 0:BL]
    po = ph2[:BL, 0:F]

    v12_b = bass.AP(v12.tensor, v12.offset, [list(v12.ap[0]), [0, PBATCH], [1, NH]])

    sdh = [nc.alloc_semaphore(f"sdh{c}") for c in range(NPAIR)]
    sdr = nc.alloc_semaphore("sdr")
    sdw = nc.alloc_semaphore("sdw")
    sdf = nc.alloc_semaphore("sdf")
    sp = nc.alloc_semaphore("sp")
    sa = nc.alloc_semaphore("sa")
    sv = nc.alloc_semaphore("sv")
    sg = nc.alloc_semaphore("sg")
    sq = nc.alloc_semaphore("sq")
    all_sems = sdh + [sdr, sdw, sdf, sp, sa, sv, sg]

    PC = 1008

    def ping(p):
        return (p % 2) * 1024

    def pair_b(p):
        return slice(p * PBATCH, (p + 1) * PBATCH)

    def ph_in(ph, p):
        o = ping(p)
        return bass.AP(ph.tensor, ph.offset + o, [list(ph.ap[0]), [512, 2], [1, SLG]])

    with nc.Block(no_gpsimd_drain=True) as block:

        @block.sync
        def _(sync):
            for c in range(NPAIR):
                sync.dma_start(
                    out=hTa[:, c * PC: (c + 1) * PC],
                    in_=d_hTa[:, c * PC: (c + 1) * PC],
                ).then_inc(sdh[c], 16)
            sync.wait_ge(sv, 8)
            sync.dma_start(out=d_out[:], in_=out_sb[:]).then_inc(sq, 16)

        @block.tensor
        def _(tensor):
            def l1(p, *waits):
                for s, v in waits:
                    tensor.wait_ge(s, v)
                o = ping(p)
                tensor.matmul(ph1[:, o: o + SLG], wh1a, hTa[:, p * PC: p * PC + SLG],
                              start=True, stop=True)
                tensor.matmul(ph1[:, o + 512: o + 512 + SLG], wh1a,
                              hTa[:, p * PC + SLG: (p + 1) * PC],
                              start=True, stop=True).then_inc(sp)

            def l2(p, *waits):
                for s, v in waits:
                    tensor.wait_ge(s, v)
                o = ping(p)
                tensor.matmul(ph2[:, o: o + SLG], wh2d, h1[:, p * PC: p * PC + SLG],
                              start=True, stop=True)
                tensor.matmul(ph2[:, o + 512: o + 512 + SLG], wh2d,
                              h1[:, p * PC + SLG: (p + 1) * PC],
                              start=True, stop=True).then_inc(sp)

            tensor.wait_ge(sdr, 16)
            tensor.matmul(pr1, wr1a, rTa, start=True, stop=True).then_inc(sp)
            tensor.wait_ge(sa, 1)
            tensor.matmul(pr2, wr2, r1[:], start=True, stop=True).then_inc(sp)
            tensor.wait_ge(sdw, 16)
            l1(0, (sdh[0], 16), (sa, 1))
            l1(1, (sdh[1], 16))
            l2(0, (sa, 3))
            l1(2, (sdh[2], 16), (sa, 3))
            l2(1, (sa, 4))
            l1(3, (sdh[3], 16), (sa, 4))
            l2(2, (sa, 6))
            l2(3, (sa, 8))
            tensor.wait_ge(sv, 7)
            tensor.wait_ge(sa, 9)
            tensor.wait_ge(sdf, 16)
            tensor.matmul(po, c12[:], W12, start=True, stop=False)
            tensor.matmul(po, r2[:], Ar, start=False, stop=False)
            tensor.matmul(po, onesr, chebb, start=False, stop=True).then_inc(sp)

        @block.scalar
        def _(scalar):
            scalar.dma_start(out=pa[:], in_=d_pa[:]).then_inc(sdr, 16)
            scalar.dma_start(out=pr_[:], in_=d_pr[:]).then_inc(sdw, 16)
            scalar.dma_start(out=pb[:], in_=d_pb[:]).then_inc(sdf, 16)
            scalar.wait_ge(sp, 1)
            scalar.activation(r1[:], pr1, AF.Relu).then_inc(sa)
            scalar.wait_ge(sp, 2)
            scalar.activation(r2[:], pr2, AF.Relu, bias=br2).then_inc(sa)
            scalar.wait_ge(sp, 3)
            scalar.activation(h1[:, 0:PC], ph_in(ph1, 0), AF.Relu).then_inc(sa)
            scalar.wait_ge(sp, 4)
            scalar.activation(h1[:, PC: 2 * PC], ph_in(ph1, 1), AF.Relu).then_inc(sa)
            scalar.wait_ge(sp, 5)
            scalar.wait_ge(sdf, 16)
            scalar.activation(h2d[:, pair_b(0), :], ph_in(ph2, 0), AF.Relu,
                              bias=bh2d).then_inc(sa)
            scalar.wait_ge(sp, 6)
            scalar.activation(h1[:, 2 * PC: 3 * PC], ph_in(ph1, 2), AF.Relu).then_inc(sa)
            scalar.wait_ge(sp, 7)
            scalar.activation(h2d[:, pair_b(1), :], ph_in(ph2, 1), AF.Relu,
                              bias=bh2d).then_inc(sa)
            scalar.wait_ge(sp, 8)
            scalar.activation(h1[:, 3 * PC: 4 * PC], ph_in(ph1, 3), AF.Relu).then_inc(sa)
            scalar.wait_ge(sp, 9)
            scalar.activation(h2d[:, pair_b(2), :], ph_in(ph2, 2), AF.Relu,
                              bias=bh2d).then_inc(sa)
            scalar.wait_ge(sp, 10)
            scalar.activation(h2d[:, pair_b(3), :], ph_in(ph2, 3), AF.Relu,
                              bias=bh2d).then_inc(sa)

        @block.vector
        def _(vector):
            def red(p, *waits):
                for s, v in waits:
                    vector.wait_ge(s, v)
                vector.tensor_reduce(c12[:, pair_b(p)], tmp[:, pair_b(p), :],
                                     axis=AX.X, op=ALU.add).then_inc(sv)

            vector.wait_ge(sa, 5)
            vector.wait_ge(sdf, 16)
            vector.tensor_tensor(tmp[:, pair_b(0), :], h2d[:, pair_b(0), :],
                                 v12_b, ALU.mult).then_inc(sv)
            red(0, (sv, 1))
            vector.wait_ge(sa, 7)
            vector.tensor_tensor(tmp[:, pair_b(1), :], h2d[:, pair_b(1), :],
                                 v12_b, ALU.mult).then_inc(sv)
            red(1, (sv, 3))
            vector.wait_ge(sa, 10)
            vector.tensor_tensor(tmp[:, pair_b(3), :], h2d[:, pair_b(3), :],
                                 v12_b, ALU.mult).then_inc(sv)
            red(3, (sv, 5))
            red(2, (sg, 1))
            vector.wait_ge(sp, 11)
            vector.tensor_copy(out_sb[:], po).then_inc(sv)

        @block.gpsimd
        def _(gpsimd):
            gpsimd.wait_ge(sa, 9)
            gpsimd.tensor_tensor(tmp[:, pair_b(2), :], h2d[:, pair_b(2), :],
                                 v12_b, ALU.mult).then_inc(sg)

    nc.clear_and_free_semaphores(all_sems)
    nc.compile()
    return nc


def _host_prep_general(v1, v2, robot_x, human_x, edge_index, wr1_w, wr1_b, wr2_w,
                       wr2_b, wh1_w, wh1_b, wh2_w, wh2_b, cheb_w, cheb_b):
    f32 = np.float32
    robot_x = np.ascontiguousarray(np.asarray(robot_x, f32))
    human_x = np.ascontiguousarray(np.asarray(human_x, f32))
    W0, W1, W2 = (np.asarray(cheb_w, f32)[k] for k in range(3))
    wh1_w = np.asarray(wh1_w, f32); wh1_b = np.asarray(wh1_b, f32)
    wh2_w = np.asarray(wh2_w, f32); wh2_b = np.asarray(wh2_b, f32)
    wr1_w = np.asarray(wr1_w, f32); wr1_b = np.asarray(wr1_b, f32)
    wr2_w = np.asarray(wr2_w, f32); wr2_b = np.asarray(wr2_b, f32)
    cheb_b = np.asarray(cheb_b, f32)

    pa = np.zeros((HID, 257), f32)
    pa[:, 0:64] = wr2_w
    pa[0:64, 64] = wr2_b
    pa[0:ROBOT_DIM, 129:257] = wr1_w
    pa[ROBOT_DIM, 129:257] = wr1_b
    pb = np.zeros((HID, 320), f32)
    pb[0:64, 0:64] = W1
    pb[64:128, 0:64] = f32(2.0) * W2
    pb[0:64, 64:127] = np.tile(v1[1:], (F, 1))
    pb[64:128, 64:127] = np.tile(v2[1:], (F, 1))
    pb[0:64, 127] = wh2_b
    pb[64:128, 127] = wh2_b
    pb[0:64, 128:192] = W0 - W2 + v1[0] * W1 + f32(2.0) * v2[0] * W2
    pb[0, 192:256] = f32(1.0)
    pb[0, 256:320] = cheb_b
    pr = np.zeros((HID, 256), f32)
    pr[:, 0:128] = np.hstack([wh2_w, wh2_w])
    pr[0:HUMAN_DIM, 128:256] = wh1_w
    pr[HUMAN_DIM, 128:256] = wh1_b
    shared = {"pa": pa, "pb": pb, "pr": pr}

    in_maps = []
    ones_tok = np.ones((1, TOK), f32)
    for c in range(NCORES):
        bs = slice(c * BL, (c + 1) * BL)
        hT = human_x[bs].transpose(2, 0, 1).reshape(HUMAN_DIM, TOK)
        m = dict(shared)
        m["hTa"] = np.ascontiguousarray(np.vstack([hT, ones_tok]))
        pac = shared["pa"].copy()
        pac[0:ROBOT_DIM, 65:129] = robot_x[bs, 0, :].T
        pac[ROBOT_DIM, 65:129] = f32(1.0)
        m["pa"] = pac
        in_maps.append(m)
    return in_maps


# --------------------------------------------------------------------------
# driver
# --------------------------------------------------------------------------

def run(inputs, trace=False, tmpdir=None):
    """Run the Bass kernel on 8 cores. Returns (full_output, BassKernelResults)."""
    from concourse.bass_utils import run_bass_kernel_spmd

    v1, v2 = _graph_vectors(inputs["edge_index"])
    const_ok = (np.allclose(v1[1:], v1[1], rtol=1e-5, atol=1e-7)
                and np.allclose(v2[1:], v2[1], rtol=1e-5, atol=1e-7))

    key = "const" if const_ok else "general"
    if key not in _STATE:
        _STATE[key] = _build_bass_const() if const_ok else _build_bass_general()
    nc = _STATE[key]

    if const_ok:
        in_maps = _host_prep_const(v1, v2, **inputs)
    else:
        in_maps = _host_prep_general(v1, v2, **inputs)
    res = run_bass_kernel_spmd(
        nc, in_maps, list(range(NCORES)), trace=trace, tmpdir=tmpdir
    )
    out = np.concatenate([res.results[c]["out"] for c in range(NCORES)], axis=0)
    return np.ascontiguousarray(out, dtype=np.float32), res


def kernel(**inputs) -> np.ndarray:
    out, _ = run(inputs, trace=False)
    return out
